# revision 18
# baseline (speedup 1.0000x reference)
"""Trainium2 Bass kernel for: Conv3d(3->16, k=3, VALID) -> min over depth -> softmax(channels).

Full inputs:  x [8, 3, 24, 128, 128] f32, conv_weight [16, 3, 3, 3, 3] f32
Full output:  [8, 16, 126, 126] f32
Sharding: data-parallel over batch, one sample per NeuronCore (8 cores).

Per-core scheme (H-packed im2col, fp16 matmuls, K padded to 128):
  - 16 h-blocks b; block handles h_out in [8b, 8b+8) (last block: 6 rows).
  - Host pre-packs x into x5 [16, 128, 24, 126] fp16:
    x5[b, (c*3+kw)*hh_n + hh, d, w] = x[c, d, 8b+hh, kw+w], rows 90.. zero.
    One fully-contiguous 128-partition DMA per block.
  - lhsT_p [128, M=(h_l,oc)] = W[oc, c, p, hh-h_l, kw] (0 <= hh-h_l < 3), p=kd.
  - Depth quads (4,4,4 | 4,3,3); per quad 3 kd passes accumulate in PSUM
    (start/stop flags). PSUM layout: two 3-bank tiles QA/QB [128, 3, 512].
  - min over depth, split across engines (the v1 kernel was DVE-bound at
    ~80us busy; PE busy is ~56.5us, so everything else must fit under it):
      Act:  copies quads 0-3 (14 depths) PSUM -> SBUF fp16 (Copy
            activation = downcast; ~2.1us/block).
      DVE:  X-min-reduces quads 4-5 straight from PSUM (fp32 at 1
            elem/cycle - TensorReduce has no 2x mode; ~1.3us/block), plus
            a pairwise fp16 min tree over the Act stage (TensorTensor DOES
            have the 2x_1p mode => half cost per element; ~1.4us/block).
            The tree for block b runs during block b+1 so DVE's in-order
            queue never delays the PSUM reduces that PE waits on for
            bank recycling.
      (GpSimd/Pool is unusable: no PSUM port, and walrus rejects
      TensorTensor on Pool outright. Act can't min. So DVE must see
      every depth value once, fp32 from PSUM or fp16 post-copy.)
  - PSUM: one tile (bank) per quad, bufs=6 + 2 st banks: every bank
    recycles the moment its consumer read it. Single big multi-bank
    tiles stall PE ~1us/block on the coarse dependencies.
  - quad pairs share each kd Ldweights (9 loads/block, not 18).
  - softmax over the 16 channels per partition group of 16, as
    exp(mn - ln(sum exp)): the channel-sum-and-broadcast is ONE fp16 PE
    matmul with a 0/1 block matrix ob[k,p] = 1 iff k//16 == p//16;
    mn - ln on DVE (fp16 2x). Exp/Ln/Copy all live in act table 6
    (natural_log_exp_and_others), pre-loaded explicitly - otherwise the
    table chooser thrashes tables 0/5 (11 x 1283ns LoadActFuncSet).
  - Per-group softmax steps are emitted AFTER each block's copies
    (Act in-order queue never head-blocks the copies the DVE tree
    needs), spaced 2..6 blocks after the group's mn completes.
  - Output rides per-GROUP DMAs (5 total) on the SP queue into a packed
    dram tensor yp [128, 16, 126] fp16; host unshuffles/upcasts (free).
  Steady state simulates at ~56.4us/sample marginal = 99.8% PE
  occupancy; HW measures ~57.8us with a 32-body-unrolled timing loop.
"""

import functools
import os
import sys

import numpy as np

os.environ.setdefault("MYCRO_LOCAL_CACHE", "1")
if os.path.isdir("/opt/trn_rl_repo") and "/opt/trn_rl_repo" not in sys.path:
    sys.path.insert(0, "/opt/trn_rl_repo")

import concourse.bacc as bacc
import concourse.mybir as mybir
import concourse.tile as tile
from concourse import bass_utils

C, D, H, W = 3, 24, 128, 128
OC, KD, KH, KW = 16, 3, 3, 3
DO, HO, WO = D - 2, H - 2, W - 2  # 22, 126, 126
# depth quads (start, ndepth): first 4 go to the Act-copied QB banks (14
# depths), last 2 to the DVE-reduced QA banks (8 depths)
DQ = ((0, 4), (4, 4), (8, 3), (11, 3), (14, 4), (18, 4))
NCORES = 8
NBLK = 16  # h blocks: 15 full (8 rows) + 1 tail (6 rows)
GROUPS = ((0, 4), (4, 4), (8, 4), (12, 3), (15, 1))  # softmax groups (start, size)
F32 = mybir.dt.float32
F16 = mybir.dt.float16
AF = mybir.ActivationFunctionType


def _pack_weights(w: np.ndarray):
    """lhsT/lhsT_last [KD,128,128] (zero-padded K and M) + ob [128,128]."""

    def pack(nh):
        hh_n = nh + 2
        lhsT = np.zeros((KD, 128, 128), dtype=np.float32)
        for p in range(KD):
            for c in range(C):
                for kw in range(KW):
                    for hh in range(hh_n):
                        r = (c * KW + kw) * hh_n + hh
                        for hl in range(nh):
                            kh = hh - hl
                            if 0 <= kh < KH:
                                lhsT[p, r, hl * OC : hl * OC + OC] = w[:, c, p, kh, kw]
        return lhsT

    ob = np.zeros((128, 128), dtype=np.float32)
    for pp in range(128):
        g0 = (pp // OC) * OC
        ob[pp, g0 : g0 + OC] = 1.0
    return pack(8), pack(6), ob


def _pack_x5(x1: np.ndarray) -> np.ndarray:
    """x [3,24,128,128] f32 -> x5 [NBLK,128,24,126] f16 (padded rows zero)."""
    x5 = np.zeros((NBLK, 128, D, WO), dtype=np.float16)
    for b in range(NBLK):
        nh = 8 if b < NBLK - 1 else 6
        hh_n = nh + 2
        for c in range(C):
            for kw in range(KW):
                r0 = (c * KW + kw) * hh_n
                # [hh, d, w] <- x[c, d, 8b+hh, kw+w]
                x5[b, r0 : r0 + hh_n] = np.transpose(
                    x1[c, :, 8 * b : 8 * b + hh_n, kw : kw + WO], (1, 0, 2)
                )
    return x5


def build_program(reps: int = 1, stage2: str = "full", unroll: int = 1):
    """reps > 1 wraps the per-sample body in a hardware loop (dev timing only).
    unroll > 1 emits the body N times with no loop (dev: cross-rep pipelining).
    stage2: none | exp | smmm | full (dev bisection of the softmax tail)."""
    nc = bacc.Bacc(
        "TRN2",
        target_bir_lowering=False,
        debug=False,
        enable_asserts=True,
        num_devices=NCORES,
    )
    x5_d = nc.dram_tensor("x5", [NBLK, 128, D, WO], F16, kind="ExternalInput").ap()
    lw_d = nc.dram_tensor("lw", [KD, 128, 128], F16, kind="ExternalInput").ap()
    lwl_d = nc.dram_tensor("lwl", [KD, 128, 128], F16, kind="ExternalInput").ap()
    ob_d = nc.dram_tensor("ob", [128, 128], F16, kind="ExternalInput").ap()
    yp_d = nc.dram_tensor("yp", [128, NBLK, WO], F16, kind="ExternalOutput").ap()

    with tile.TileContext(nc) as tc:
        with (
            tc.tile_pool(name="const", bufs=1) as cpool,
            tc.tile_pool(name="xt", bufs=6) as xpool,
            tc.tile_pool(name="sm", bufs=3) as spool,
            tc.tile_pool(name="qps", bufs=1, space="PSUM") as qpool,
            tc.tile_pool(name="sps", bufs=2, space="PSUM") as smpool,
        ):
            lw_sb = cpool.tile([128, KD, 128], F16)
            nc.sync.dma_start(lw_sb[:], lw_d.rearrange("p r m -> r p m").bitcast(F16))
            lwl_sb = cpool.tile([128, KD, 128], F16)
            nc.sync.dma_start(lwl_sb[:], lwl_d.rearrange("p r m -> r p m").bitcast(F16))
            ob_sb = cpool.tile([128, 128], F16)
            nc.sync.dma_start(ob_sb[:], ob_d)

            # Pre-place the combined exp+ln+copy act table (set 6,
            # natural_log_exp_and_others). Without this the table chooser
            # alternates exp_and_others <-> natural_log, inserting 11
            # LoadActFuncSet (1283ns each) that stall the Act queue.
            lset = mybir.InstLoadActFuncSet(
                name=nc.get_next_instruction_name(), act_func_set_id=6
            )
            lset.engine = mybir.EngineType.Activation
            nc.add_instruction(lset)

            def emit_body():
                state = {}  # per softmax group g: mn/et/st/lt/dt/ot tiles

                def softmax_step(step, g):
                    g0, gsz = GROUPS[g]
                    if step == 0 and stage2 != "none":
                        et = spool.tile([128, gsz, WO], F16, tag="et", bufs=2, name=f"et{g}")
                        nc.scalar.activation(et[:], state[g]["mn"][:], AF.Exp)
                        state[g]["et"] = et
                    if stage2 in ("none", "exp"):
                        return
                    if step == 1:
                        # group-sum broadcast to all 128 partitions in one MM:
                        # ob[k, p] = 1 iff k//16 == p//16
                        st = smpool.tile([128, gsz, WO], F32, tag="ss", name=f"st{g}")
                        nc.tensor.matmul(st[:], ob_sb[:], state[g]["et"][:], start=True, stop=True)
                        state[g]["st"] = st
                    elif step == 2:
                        lt = spool.tile([128, gsz, WO], F16, tag="lt", bufs=2, name=f"lt{g}")
                        nc.scalar.activation(lt[:], state[g]["st"][:], AF.Ln)
                        state[g]["lt"] = lt
                    elif step == 3:
                        dt = spool.tile([128, gsz, WO], F16, tag="dt", bufs=2, name=f"dt{g}")
                        nc.vector.tensor_tensor(
                            dt[:], state[g]["mn"][:], state[g]["lt"][:],
                            op=mybir.AluOpType.subtract,
                        )
                        state[g]["dt"] = dt
                    elif step == 4:
                        if stage2 == "smmm":
                            return
                        ot = spool.tile([128, gsz, WO], F16, tag="ot", bufs=2, name=f"ot{g}")
                        nc.scalar.activation(ot[:], state[g]["dt"][:], AF.Exp)
                        # output rides the SP queue: SP only issues DMAs, so
                        # this never competes with Act's copy/softmax work
                        nc.sync.dma_start(yp_d[:, g0 : g0 + gsz, :], ot[:])

                # schedule[B] = list of (step, g) to emit during conv block B.
                # mn(g) completes one block after the group's last conv block
                # (the QB tree is software-pipelined one block behind), so the
                # chain starts at end+2.
                schedule = {}
                for g, (g0, gsz) in enumerate(GROUPS):
                    end = g0 + gsz - 1
                    for step, off in enumerate((2, 3, 4, 5, 6)):
                        schedule.setdefault(end + off, []).append((step, g))

                blk2grp = {}
                for g, (g0, gsz) in enumerate(GROUPS):
                    for b in range(g0, g0 + gsz):
                        blk2grp[b] = g

                def emit_tree(prev):
                    """Fold block prev's QB stage (14 fp16 depths, 2x mode)
                    + its QA reduce into its mn slice. Runs one block late so
                    DVE's in-order queue never delays the PSUM reduce."""
                    qc = prev["qc"]
                    u = spool.tile([128, 7, WO], F16, tag="u", bufs=2)
                    nc.vector.tensor_tensor(u[:], qc[:, 0:7, :], qc[:, 7:14, :], op=mybir.AluOpType.min)
                    v = spool.tile([128, 3, WO], F16, tag="v", bufs=2)
                    nc.vector.tensor_tensor(v[:], u[:, 0:3, :], u[:, 3:6, :], op=mybir.AluOpType.min)
                    w2 = spool.tile([128, WO], F16, tag="w2", bufs=2)
                    nc.vector.tensor_tensor(w2[:], v[:, 0, :], v[:, 1, :], op=mybir.AluOpType.min)
                    x2 = spool.tile([128, WO], F16, tag="x2", bufs=2)
                    nc.vector.tensor_tensor(x2[:], w2[:], v[:, 2, :], op=mybir.AluOpType.min)
                    t5 = spool.tile([128, WO], F16, tag="t5", bufs=2)
                    nc.vector.tensor_tensor(t5[:], x2[:], u[:, 6, :], op=mybir.AluOpType.min)
                    ra = spool.tile([128, WO], F16, tag="ra", bufs=2)
                    nc.vector.tensor_tensor(
                        ra[:], prev["r4"][:], prev["r5"][:], op=mybir.AluOpType.min
                    )
                    nc.vector.tensor_tensor(
                        prev["mn"], t5[:], ra[:], op=mybir.AluOpType.min
                    )

                prev = None
                for b in range(NBLK):
                    g_cur = blk2grp[b]
                    g0, gsz = GROUPS[g_cur]
                    if b == g0:
                        state[g_cur] = {
                            "mn": spool.tile([128, gsz, WO], F16, tag="mn", bufs=3, name=f"mn{g_cur}")
                        }
                    lw_t = lw_sb if b < NBLK - 1 else lwl_sb

                    xt = xpool.tile([128, D, WO], F16, tag="xt")
                    nc.sync.dma_start(xt[:], x5_d[b].bitcast(F16))

                    # conv PSUM: one tile (= one bank) per quad, bufs=6, so
                    # every bank recycles independently the moment ITS
                    # consumer has read it (max scheduling freedom). Quads
                    # 0-3 are Act-copied (cheap, early); quads 4-5 are DVE
                    # X-reduced and consumed by next block's LAST matmuls.
                    cur = {"mn": state[g_cur]["mn"][:, b - g0, :]}
                    qc_t = spool.tile([128, 14, WO], F16, tag="qc", bufs=3)
                    cur["qc"] = qc_t
                    qc_lo = (0, 4, 8, 11)
                    # quads run in pairs sharing each kd weight-load (9
                    # Ldweights per block instead of 18); stops still land
                    # per-quad-pair so consumers fire mid-block
                    for pr in range(3):
                        qs = (2 * pr, 2 * pr + 1)
                        pts = [
                            qpool.tile([128, DQ[qi][1], WO], F32, tag="q", bufs=6, name=f"pt{qi}")
                            for qi in qs
                        ]
                        for p in range(KD):
                            for pt, qi in zip(pts, qs):
                                dq, nd = DQ[qi]
                                nc.tensor.matmul(
                                    pt[:],
                                    lw_t[:, p, :],
                                    xt[:, dq + p : dq + p + nd, :],
                                    start=(p == 0),
                                    stop=(p == KD - 1),
                                )
                        for pt, qi in zip(pts, qs):
                            if qi < 4:
                                # Act stages quads 0-3 into SBUF fp16 (cheap
                                # downcast Copy; emitted before any softmax Act
                                # steps so the Act queue never head-blocks)
                                lo = qc_lo[qi]
                                nc.scalar.activation(
                                    cur["qc"][:, lo : lo + DQ[qi][1], :], pt[:], AF.Copy
                                )
                            else:
                                # DVE min-reduce straight from PSUM (1 elem/cyc)
                                rq = spool.tile([128, WO], F16, tag=f"r{qi}", bufs=2)
                                nc.vector.tensor_reduce(
                                    rq[:],
                                    pt[:].rearrange("m j w -> m w j"),
                                    axis=mybir.AxisListType.X,
                                    op=mybir.AluOpType.min,
                                )
                                cur[f"r{qi}"] = rq

                    if prev is not None:
                        emit_tree(prev)
                    prev = cur

                    # softmax steps last: their Act/DVE/PE ops queue behind
                    # this block's copies/reduces, with deps long satisfied
                    for step, g in schedule.get(b, []):
                        softmax_step(step, g)

                # drain: tree for the last block, then remaining softmax steps
                emit_tree(prev)
                for at in sorted(k for k in schedule if k >= NBLK):
                    for step, g in schedule[at]:
                        softmax_step(step, g)

            if reps == 1:
                for _ in range(unroll):
                    emit_body()
            else:
                # reps = loop iterations; each iteration runs `unroll` bodies
                # (amortizes the For_i all-engine-barrier drain/fill, ~14us,
                # across `unroll` samples)
                with tc.For_i(0, reps, 1, hint_engines=(mybir.EngineType.PE,), staggered_reset=True):
                    for _ in range(unroll):
                        emit_body()

    nc.compile()
    return nc


@functools.lru_cache(maxsize=1)
def _program():
    return build_program()


def make_in_maps(x: np.ndarray, w: np.ndarray):
    lw, lwl, ob = _pack_weights(w)
    lw = lw.astype(np.float16)
    lwl = lwl.astype(np.float16)
    return [
        {"x5": _pack_x5(x[i]), "lw": lw, "lwl": lwl, "ob": ob.astype(np.float16)}
        for i in range(x.shape[0])
    ]


def _unpack_yp(yp: np.ndarray) -> np.ndarray:
    """yp [128, 16, 126] fp16 -> y [16, 126, 126] f32."""
    v = yp.reshape(8, OC, NBLK, WO)  # [hl, oc, b, w]
    y = np.transpose(v, (1, 2, 0, 3)).reshape(OC, NBLK * 8, WO)  # [oc, 8b+hl, w]
    return y[:, :HO, :].astype(np.float32)


def kernel(x, conv_weight):
    x = np.ascontiguousarray(np.asarray(x, dtype=np.float32))
    w = np.ascontiguousarray(np.asarray(conv_weight, dtype=np.float32))
    assert x.shape == (NCORES, C, D, H, W), x.shape
    nc = _program()
    in_maps = make_in_maps(x, w)
    res = bass_utils.run_bass_kernel_spmd(nc, in_maps, core_ids=list(range(NCORES)))
    out = np.stack([_unpack_yp(res.results[i]["yp"]) for i in range(NCORES)])
    return out


# revision 19
# speedup vs baseline: 1.0049x; 1.0049x over previous
"""Trainium2 Bass kernel for: Conv3d(3->16, k=3, VALID) -> min over depth -> softmax(channels).

Full inputs:  x [8, 3, 24, 128, 128] f32, conv_weight [16, 3, 3, 3, 3] f32
Full output:  [8, 16, 126, 126] f32
Sharding: data-parallel over batch, one sample per NeuronCore (8 cores).

Per-core scheme (H-packed im2col, fp16 matmuls, K padded to 128):
  - 16 h-blocks b; block handles h_out in [8b, 8b+8) (last block: 6 rows).
  - Host pre-packs x into x5 [16, 128, 24, 126] fp16:
    x5[b, (c*3+kw)*hh_n + hh, d, w] = x[c, d, 8b+hh, kw+w], rows 90.. zero.
    One fully-contiguous 128-partition DMA per block.
  - lhsT_p [128, M=(h_l,oc)] = W[oc, c, p, hh-h_l, kw] (0 <= hh-h_l < 3), p=kd.
  - Depth quads (4,4,4 | 4,3,3); per quad 3 kd passes accumulate in PSUM
    (start/stop flags). PSUM layout: two 3-bank tiles QA/QB [128, 3, 512].
  - min over depth, split across engines (the v1 kernel was DVE-bound at
    ~80us busy; PE busy is ~56.5us, so everything else must fit under it):
      Act:  copies quads 0-3 (14 depths) PSUM -> SBUF fp16 (Copy
            activation = downcast; ~2.1us/block).
      DVE:  X-min-reduces quads 4-5 straight from PSUM (fp32 at 1
            elem/cycle - TensorReduce has no 2x mode; ~1.3us/block), plus
            a pairwise fp16 min tree over the Act stage (TensorTensor DOES
            have the 2x_1p mode => half cost per element; ~1.4us/block).
            The tree for block b runs during block b+1 so DVE's in-order
            queue never delays the PSUM reduces that PE waits on for
            bank recycling.
      (GpSimd/Pool is unusable: no PSUM port, and walrus rejects
      TensorTensor on Pool outright. Act can't min. So DVE must see
      every depth value once, fp32 from PSUM or fp16 post-copy.)
  - PSUM: one tile (bank) per quad, bufs=6 + 2 st banks: every bank
    recycles the moment its consumer read it. Single big multi-bank
    tiles stall PE ~1us/block on the coarse dependencies.
  - quad pairs share each kd Ldweights (9 loads/block, not 18).
  - softmax over the 16 channels per partition group of 16, as
    exp(mn - ln(sum exp)): the channel-sum-and-broadcast is ONE fp16 PE
    matmul with a 0/1 block matrix ob[k,p] = 1 iff k//16 == p//16;
    mn - ln on DVE (fp16 2x). Exp/Ln/Copy all live in act table 6
    (natural_log_exp_and_others), pre-loaded explicitly - otherwise the
    table chooser thrashes tables 0/5 (11 x 1283ns LoadActFuncSet).
  - Per-group softmax steps are emitted AFTER each block's copies
    (Act in-order queue never head-blocks the copies the DVE tree
    needs), spaced 2..6 blocks after the group's mn completes.
  - Output rides per-GROUP DMAs (5 total) on the SP queue into a packed
    dram tensor yp [128, 16, 126] fp16; host unshuffles/upcasts (free).
  Steady state simulates at ~56.4us/sample marginal = 99.8% PE
  occupancy; HW measures ~57.8us with a 32-body-unrolled timing loop.
"""

import functools
import os
import sys

import numpy as np

os.environ.setdefault("MYCRO_LOCAL_CACHE", "1")
if os.path.isdir("/opt/trn_rl_repo") and "/opt/trn_rl_repo" not in sys.path:
    sys.path.insert(0, "/opt/trn_rl_repo")

import concourse.bacc as bacc
import concourse.mybir as mybir
import concourse.tile as tile
from concourse import bass_utils

C, D, H, W = 3, 24, 128, 128
OC, KD, KH, KW = 16, 3, 3, 3
DO, HO, WO = D - 2, H - 2, W - 2  # 22, 126, 126
# depth quads (start, ndepth): first 4 go to the Act-copied QB banks (14
# depths), last 2 to the DVE-reduced QA banks (8 depths)
DQ = ((0, 4), (4, 4), (8, 3), (11, 3), (14, 4), (18, 4))
NCORES = 8
NBLK = 16  # h blocks: 15 full (8 rows) + 1 tail (6 rows)
GROUPS = ((0, 4), (4, 4), (8, 4), (12, 3), (15, 1))  # softmax groups (start, size)
F32 = mybir.dt.float32
F16 = mybir.dt.float16
AF = mybir.ActivationFunctionType


def _pack_weights(w: np.ndarray):
    """lhsT/lhsT_last [KD,128,128] (zero-padded K and M) + ob [128,128]."""

    def pack(nh):
        hh_n = nh + 2
        lhsT = np.zeros((KD, 128, 128), dtype=np.float32)
        for p in range(KD):
            for c in range(C):
                for kw in range(KW):
                    for hh in range(hh_n):
                        r = (c * KW + kw) * hh_n + hh
                        for hl in range(nh):
                            kh = hh - hl
                            if 0 <= kh < KH:
                                lhsT[p, r, hl * OC : hl * OC + OC] = w[:, c, p, kh, kw]
        return lhsT

    ob = np.zeros((128, 128), dtype=np.float32)
    for pp in range(128):
        g0 = (pp // OC) * OC
        ob[pp, g0 : g0 + OC] = 1.0
    return pack(8), pack(6), ob


def _pack_x5(x1: np.ndarray) -> np.ndarray:
    """x [3,24,128,128] f32 -> x5 [NBLK,128,24,126] f16 (padded rows zero)."""
    x5 = np.zeros((NBLK, 128, D, WO), dtype=np.float16)
    for b in range(NBLK):
        nh = 8 if b < NBLK - 1 else 6
        hh_n = nh + 2
        for c in range(C):
            for kw in range(KW):
                r0 = (c * KW + kw) * hh_n
                # [hh, d, w] <- x[c, d, 8b+hh, kw+w]
                x5[b, r0 : r0 + hh_n] = np.transpose(
                    x1[c, :, 8 * b : 8 * b + hh_n, kw : kw + WO], (1, 0, 2)
                )
    return x5


def build_program(reps: int = 1, stage2: str = "full", unroll: int = 1):
    """reps > 1 wraps the per-sample body in a hardware loop (dev timing only).
    unroll > 1 emits the body N times with no loop (dev: cross-rep pipelining).
    stage2: none | exp | smmm | full (dev bisection of the softmax tail)."""
    nc = bacc.Bacc(
        "TRN2",
        target_bir_lowering=False,
        debug=False,
        enable_asserts=True,
        num_devices=NCORES,
    )
    x5_d = nc.dram_tensor("x5", [NBLK, 128, D, WO], F16, kind="ExternalInput").ap()
    lw_d = nc.dram_tensor("lw", [KD, 128, 128], F16, kind="ExternalInput").ap()
    lwl_d = nc.dram_tensor("lwl", [KD, 128, 128], F16, kind="ExternalInput").ap()
    ob_d = nc.dram_tensor("ob", [128, 128], F16, kind="ExternalInput").ap()
    yp_d = nc.dram_tensor("yp", [128, NBLK, WO], F16, kind="ExternalOutput").ap()

    with tile.TileContext(nc) as tc:
        with (
            tc.tile_pool(name="const", bufs=1) as cpool,
            tc.tile_pool(name="xt", bufs=8) as xpool,
            tc.tile_pool(name="sm", bufs=3) as spool,
            tc.tile_pool(name="qps", bufs=1, space="PSUM") as qpool,
            tc.tile_pool(name="sps", bufs=2, space="PSUM") as smpool,
        ):
            lw_sb = cpool.tile([128, KD, 128], F16)
            nc.sync.dma_start(lw_sb[:], lw_d.rearrange("p r m -> r p m").bitcast(F16))
            lwl_sb = cpool.tile([128, KD, 128], F16)
            nc.sync.dma_start(lwl_sb[:], lwl_d.rearrange("p r m -> r p m").bitcast(F16))
            ob_sb = cpool.tile([128, 128], F16)
            nc.sync.dma_start(ob_sb[:], ob_d)

            # Pre-place the combined exp+ln+copy act table (set 6,
            # natural_log_exp_and_others). Without this the table chooser
            # alternates exp_and_others <-> natural_log, inserting 11
            # LoadActFuncSet (1283ns each) that stall the Act queue.
            lset = mybir.InstLoadActFuncSet(
                name=nc.get_next_instruction_name(), act_func_set_id=6
            )
            lset.engine = mybir.EngineType.Activation
            nc.add_instruction(lset)

            def emit_body():
                state = {}  # per softmax group g: mn/et/st/lt/dt/ot tiles

                def softmax_step(step, g):
                    g0, gsz = GROUPS[g]
                    if step == 0 and stage2 != "none":
                        et = spool.tile([128, gsz, WO], F16, tag="et", bufs=2, name=f"et{g}")
                        nc.scalar.activation(et[:], state[g]["mn"][:], AF.Exp)
                        state[g]["et"] = et
                    if stage2 in ("none", "exp"):
                        return
                    if step == 1:
                        # group-sum broadcast to all 128 partitions in one MM:
                        # ob[k, p] = 1 iff k//16 == p//16
                        st = smpool.tile([128, gsz, WO], F32, tag="ss", name=f"st{g}")
                        nc.tensor.matmul(st[:], ob_sb[:], state[g]["et"][:], start=True, stop=True)
                        state[g]["st"] = st
                    elif step == 2:
                        lt = spool.tile([128, gsz, WO], F16, tag="lt", bufs=2, name=f"lt{g}")
                        nc.scalar.activation(lt[:], state[g]["st"][:], AF.Ln)
                        state[g]["lt"] = lt
                    elif step == 3:
                        dt = spool.tile([128, gsz, WO], F16, tag="dt", bufs=2, name=f"dt{g}")
                        nc.vector.tensor_tensor(
                            dt[:], state[g]["mn"][:], state[g]["lt"][:],
                            op=mybir.AluOpType.subtract,
                        )
                        state[g]["dt"] = dt
                    elif step == 4:
                        if stage2 == "smmm":
                            return
                        ot = spool.tile([128, gsz, WO], F16, tag="ot", bufs=2, name=f"ot{g}")
                        nc.scalar.activation(ot[:], state[g]["dt"][:], AF.Exp)
                        # output rides the SP queue: SP only issues DMAs, so
                        # this never competes with Act's copy/softmax work
                        nc.sync.dma_start(yp_d[:, g0 : g0 + gsz, :], ot[:])

                # schedule[B] = list of (step, g) to emit during conv block B.
                # mn(g) completes one block after the group's last conv block
                # (the QB tree is software-pipelined one block behind), so the
                # chain starts at end+2.
                schedule = {}
                for g, (g0, gsz) in enumerate(GROUPS):
                    end = g0 + gsz - 1
                    for step, off in enumerate((2, 3, 4, 5, 6)):
                        schedule.setdefault(end + off, []).append((step, g))

                blk2grp = {}
                for g, (g0, gsz) in enumerate(GROUPS):
                    for b in range(g0, g0 + gsz):
                        blk2grp[b] = g

                def emit_tree(prev):
                    """Fold block prev's QB stage (14 fp16 depths, 2x mode)
                    + its QA reduce into its mn slice. Runs one block late so
                    DVE's in-order queue never delays the PSUM reduce."""
                    qc = prev["qc"]
                    u = spool.tile([128, 7, WO], F16, tag="u", bufs=2)
                    nc.vector.tensor_tensor(u[:], qc[:, 0:7, :], qc[:, 7:14, :], op=mybir.AluOpType.min)
                    v = spool.tile([128, 3, WO], F16, tag="v", bufs=2)
                    nc.vector.tensor_tensor(v[:], u[:, 0:3, :], u[:, 3:6, :], op=mybir.AluOpType.min)
                    w2 = spool.tile([128, WO], F16, tag="w2", bufs=2)
                    nc.vector.tensor_tensor(w2[:], v[:, 0, :], v[:, 1, :], op=mybir.AluOpType.min)
                    x2 = spool.tile([128, WO], F16, tag="x2", bufs=2)
                    nc.vector.tensor_tensor(x2[:], w2[:], v[:, 2, :], op=mybir.AluOpType.min)
                    t5 = spool.tile([128, WO], F16, tag="t5", bufs=2)
                    nc.vector.tensor_tensor(t5[:], x2[:], u[:, 6, :], op=mybir.AluOpType.min)
                    ra = spool.tile([128, WO], F16, tag="ra", bufs=2)
                    nc.vector.tensor_tensor(
                        ra[:], prev["r4"][:], prev["r5"][:], op=mybir.AluOpType.min
                    )
                    nc.vector.tensor_tensor(
                        prev["mn"], t5[:], ra[:], op=mybir.AluOpType.min
                    )

                prev = None
                for b in range(NBLK):
                    g_cur = blk2grp[b]
                    g0, gsz = GROUPS[g_cur]
                    if b == g0:
                        state[g_cur] = {
                            "mn": spool.tile([128, gsz, WO], F16, tag="mn", bufs=3, name=f"mn{g_cur}")
                        }
                    lw_t = lw_sb if b < NBLK - 1 else lwl_sb

                    xt = xpool.tile([128, D, WO], F16, tag="xt")
                    nc.sync.dma_start(xt[:], x5_d[b].bitcast(F16))

                    # conv PSUM: one tile (= one bank) per quad, bufs=6, so
                    # every bank recycles independently the moment ITS
                    # consumer has read it (max scheduling freedom). Quads
                    # 0-3 are Act-copied (cheap, early); quads 4-5 are DVE
                    # X-reduced and consumed by next block's LAST matmuls.
                    cur = {"mn": state[g_cur]["mn"][:, b - g0, :]}
                    qc_t = spool.tile([128, 14, WO], F16, tag="qc", bufs=3)
                    cur["qc"] = qc_t
                    qc_lo = (0, 4, 8, 11)
                    # quads run in pairs sharing each kd weight-load (9
                    # Ldweights per block instead of 18); stops still land
                    # per-quad-pair so consumers fire mid-block
                    for pr in range(3):
                        qs = (2 * pr, 2 * pr + 1)
                        pts = [
                            qpool.tile([128, DQ[qi][1], WO], F32, tag="q", bufs=6, name=f"pt{qi}")
                            for qi in qs
                        ]
                        for p in range(KD):
                            for pt, qi in zip(pts, qs):
                                dq, nd = DQ[qi]
                                nc.tensor.matmul(
                                    pt[:],
                                    lw_t[:, p, :],
                                    xt[:, dq + p : dq + p + nd, :],
                                    start=(p == 0),
                                    stop=(p == KD - 1),
                                )
                        for pt, qi in zip(pts, qs):
                            if qi < 4:
                                # Act stages quads 0-3 into SBUF fp16 (cheap
                                # downcast Copy; emitted before any softmax Act
                                # steps so the Act queue never head-blocks)
                                lo = qc_lo[qi]
                                nc.scalar.activation(
                                    cur["qc"][:, lo : lo + DQ[qi][1], :], pt[:], AF.Copy
                                )
                            else:
                                # DVE min-reduce straight from PSUM (1 elem/cyc)
                                rq = spool.tile([128, WO], F16, tag=f"r{qi}", bufs=2)
                                nc.vector.tensor_reduce(
                                    rq[:],
                                    pt[:].rearrange("m j w -> m w j"),
                                    axis=mybir.AxisListType.X,
                                    op=mybir.AluOpType.min,
                                )
                                cur[f"r{qi}"] = rq

                    if prev is not None:
                        emit_tree(prev)
                    prev = cur

                    # softmax steps last: their Act/DVE/PE ops queue behind
                    # this block's copies/reduces, with deps long satisfied
                    for step, g in schedule.get(b, []):
                        softmax_step(step, g)

                # drain: tree for the last block, then remaining softmax steps
                emit_tree(prev)
                for at in sorted(k for k in schedule if k >= NBLK):
                    for step, g in schedule[at]:
                        softmax_step(step, g)

            if reps == 1:
                for _ in range(unroll):
                    emit_body()
            else:
                # reps = loop iterations; each iteration runs `unroll` bodies
                # (amortizes the For_i all-engine-barrier drain/fill, ~14us,
                # across `unroll` samples)
                with tc.For_i(0, reps, 1, hint_engines=(mybir.EngineType.PE,), staggered_reset=True):
                    for _ in range(unroll):
                        emit_body()

    nc.compile()
    return nc


@functools.lru_cache(maxsize=1)
def _program():
    return build_program()


def make_in_maps(x: np.ndarray, w: np.ndarray):
    lw, lwl, ob = _pack_weights(w)
    lw = lw.astype(np.float16)
    lwl = lwl.astype(np.float16)
    return [
        {"x5": _pack_x5(x[i]), "lw": lw, "lwl": lwl, "ob": ob.astype(np.float16)}
        for i in range(x.shape[0])
    ]


def _unpack_yp(yp: np.ndarray) -> np.ndarray:
    """yp [128, 16, 126] fp16 -> y [16, 126, 126] f32."""
    v = yp.reshape(8, OC, NBLK, WO)  # [hl, oc, b, w]
    y = np.transpose(v, (1, 2, 0, 3)).reshape(OC, NBLK * 8, WO)  # [oc, 8b+hl, w]
    return y[:, :HO, :].astype(np.float32)


def kernel(x, conv_weight):
    x = np.ascontiguousarray(np.asarray(x, dtype=np.float32))
    w = np.ascontiguousarray(np.asarray(conv_weight, dtype=np.float32))
    assert x.shape == (NCORES, C, D, H, W), x.shape
    nc = _program()
    in_maps = make_in_maps(x, w)
    res = bass_utils.run_bass_kernel_spmd(nc, in_maps, core_ids=list(range(NCORES)))
    out = np.stack([_unpack_yp(res.results[i]["yp"]) for i in range(NCORES)])
    return out


# revision 20
# speedup vs baseline: 1.0070x; 1.0021x over previous
"""Trainium2 Bass kernel for: Conv3d(3->16, k=3, VALID) -> min over depth -> softmax(channels).

Full inputs:  x [8, 3, 24, 128, 128] f32, conv_weight [16, 3, 3, 3, 3] f32
Full output:  [8, 16, 126, 126] f32
Sharding: data-parallel over batch, one sample per NeuronCore (8 cores).

Per-core scheme (H-packed im2col, fp16 matmuls, K padded to 128):
  - 16 h-blocks b; block handles h_out in [8b, 8b+8) (last block: 6 rows).
  - Host pre-packs x into x5 [16, 128, 24, 126] fp16:
    x5[b, (c*3+kw)*hh_n + hh, d, w] = x[c, d, 8b+hh, kw+w], rows 90.. zero.
    One fully-contiguous 128-partition DMA per block.
  - lhsT_p [128, M=(h_l,oc)] = W[oc, c, p, hh-h_l, kw] (0 <= hh-h_l < 3), p=kd.
  - Depth quads (4,4,4 | 4,3,3); per quad 3 kd passes accumulate in PSUM
    (start/stop flags). PSUM layout: two 3-bank tiles QA/QB [128, 3, 512].
  - min over depth, split across engines (the v1 kernel was DVE-bound at
    ~80us busy; PE busy is ~56.5us, so everything else must fit under it):
      Act:  copies quads 0-3 (14 depths) PSUM -> SBUF fp16 (Copy
            activation = downcast; ~2.1us/block).
      DVE:  X-min-reduces quads 4-5 straight from PSUM (fp32 at 1
            elem/cycle - TensorReduce has no 2x mode; ~1.3us/block), plus
            a pairwise fp16 min tree over the Act stage (TensorTensor DOES
            have the 2x_1p mode => half cost per element; ~1.4us/block).
            The tree for block b runs during block b+1 so DVE's in-order
            queue never delays the PSUM reduces that PE waits on for
            bank recycling.
      (GpSimd/Pool is unusable: no PSUM port, and walrus rejects
      TensorTensor on Pool outright. Act can't min. So DVE must see
      every depth value once, fp32 from PSUM or fp16 post-copy.)
  - PSUM: one tile (bank) per quad, bufs=6 + 2 st banks: every bank
    recycles the moment its consumer read it. Single big multi-bank
    tiles stall PE ~1us/block on the coarse dependencies.
  - quad pairs share each kd Ldweights (9 loads/block, not 18).
  - softmax over the 16 channels per partition group of 16, as
    exp(mn - ln(sum exp)): the channel-sum-and-broadcast is ONE fp16 PE
    matmul with a 0/1 block matrix ob[k,p] = 1 iff k//16 == p//16;
    mn - ln on DVE (fp16 2x). Exp/Ln/Copy all live in act table 6
    (natural_log_exp_and_others), pre-loaded explicitly - otherwise the
    table chooser thrashes tables 0/5 (11 x 1283ns LoadActFuncSet).
  - Per-group softmax steps are emitted AFTER each block's copies
    (Act in-order queue never head-blocks the copies the DVE tree
    needs), spaced 2..6 blocks after the group's mn completes.
  - Output rides per-GROUP DMAs (5 total) on the SP queue into a packed
    dram tensor yp [128, 16, 126] fp16; host unshuffles/upcasts (free).
  Steady state simulates at ~56.4us/sample marginal = 99.8% PE
  occupancy; HW measures ~57.8us with a 32-body-unrolled timing loop.
"""

import functools
import os
import sys

import numpy as np

os.environ.setdefault("MYCRO_LOCAL_CACHE", "1")
if os.path.isdir("/opt/trn_rl_repo") and "/opt/trn_rl_repo" not in sys.path:
    sys.path.insert(0, "/opt/trn_rl_repo")

import concourse.bacc as bacc
import concourse.mybir as mybir
import concourse.tile as tile
from concourse import bass_utils

C, D, H, W = 3, 24, 128, 128
OC, KD, KH, KW = 16, 3, 3, 3
DO, HO, WO = D - 2, H - 2, W - 2  # 22, 126, 126
# depth quads (start, ndepth): first 4 go to the Act-copied QB banks (14
# depths), last 2 to the DVE-reduced QA banks (8 depths)
DQ = ((0, 4), (4, 4), (8, 3), (11, 3), (14, 4), (18, 4))
NCORES = 8
NBLK = 16  # h blocks: 15 full (8 rows) + 1 tail (6 rows)
GROUPS = ((0, 4), (4, 4), (8, 4), (12, 4))  # softmax groups (start, size);
# the tail block (15, nh=6) rides the last group: its min/softmax values in
# partitions 96..127 are finite garbage (zero-padded weights) the host drops
F32 = mybir.dt.float32
F16 = mybir.dt.float16
AF = mybir.ActivationFunctionType


def _pack_weights(w: np.ndarray):
    """lhsT/lhsT_last [KD,128,128] (zero-padded K and M) + ob [128,128]."""

    def pack(nh):
        hh_n = nh + 2
        lhsT = np.zeros((KD, 128, 128), dtype=np.float32)
        for p in range(KD):
            for c in range(C):
                for kw in range(KW):
                    for hh in range(hh_n):
                        r = (c * KW + kw) * hh_n + hh
                        for hl in range(nh):
                            kh = hh - hl
                            if 0 <= kh < KH:
                                lhsT[p, r, hl * OC : hl * OC + OC] = w[:, c, p, kh, kw]
        return lhsT

    ob = np.zeros((128, 128), dtype=np.float32)
    for pp in range(128):
        g0 = (pp // OC) * OC
        ob[pp, g0 : g0 + OC] = 1.0
    return pack(8), pack(6), ob


def _pack_x5(x1: np.ndarray) -> np.ndarray:
    """x [3,24,128,128] f32 -> x5 [NBLK,128,24,126] f16 (padded rows zero)."""
    x5 = np.zeros((NBLK, 128, D, WO), dtype=np.float16)
    for b in range(NBLK):
        nh = 8 if b < NBLK - 1 else 6
        hh_n = nh + 2
        for c in range(C):
            for kw in range(KW):
                r0 = (c * KW + kw) * hh_n
                # [hh, d, w] <- x[c, d, 8b+hh, kw+w]
                x5[b, r0 : r0 + hh_n] = np.transpose(
                    x1[c, :, 8 * b : 8 * b + hh_n, kw : kw + WO], (1, 0, 2)
                )
    return x5


def build_program(reps: int = 1, stage2: str = "full", unroll: int = 1):
    """reps > 1 wraps the per-sample body in a hardware loop (dev timing only).
    unroll > 1 emits the body N times with no loop (dev: cross-rep pipelining).
    stage2: none | exp | smmm | full (dev bisection of the softmax tail)."""
    nc = bacc.Bacc(
        "TRN2",
        target_bir_lowering=False,
        debug=False,
        enable_asserts=True,
        num_devices=NCORES,
    )
    x5_d = nc.dram_tensor("x5", [NBLK, 128, D, WO], F16, kind="ExternalInput").ap()
    lw_d = nc.dram_tensor("lw", [KD, 128, 128], F16, kind="ExternalInput").ap()
    lwl_d = nc.dram_tensor("lwl", [KD, 128, 128], F16, kind="ExternalInput").ap()
    ob_d = nc.dram_tensor("ob", [128, 128], F16, kind="ExternalInput").ap()
    yp_d = nc.dram_tensor("yp", [128, NBLK, WO], F16, kind="ExternalOutput").ap()

    with tile.TileContext(nc) as tc:
        with (
            tc.tile_pool(name="const", bufs=1) as cpool,
            tc.tile_pool(name="xt", bufs=8) as xpool,
            tc.tile_pool(name="sm", bufs=3) as spool,
            tc.tile_pool(name="qps", bufs=1, space="PSUM") as qpool,
            tc.tile_pool(name="sps", bufs=2, space="PSUM") as smpool,
        ):
            lw_sb = cpool.tile([128, KD, 128], F16)
            nc.sync.dma_start(lw_sb[:], lw_d.rearrange("p r m -> r p m").bitcast(F16))
            lwl_sb = cpool.tile([128, KD, 128], F16)
            nc.sync.dma_start(lwl_sb[:], lwl_d.rearrange("p r m -> r p m").bitcast(F16))
            ob_sb = cpool.tile([128, 128], F16)
            nc.sync.dma_start(ob_sb[:], ob_d)

            # Pre-place the combined exp+ln+copy act table (set 6,
            # natural_log_exp_and_others). Without this the table chooser
            # alternates exp_and_others <-> natural_log, inserting 11
            # LoadActFuncSet (1283ns each) that stall the Act queue.
            lset = mybir.InstLoadActFuncSet(
                name=nc.get_next_instruction_name(), act_func_set_id=6
            )
            lset.engine = mybir.EngineType.Activation
            nc.add_instruction(lset)

            def emit_body():
                state = {}  # per softmax group g: mn/et/st/lt/dt/ot tiles

                def softmax_step(step, g):
                    g0, gsz = GROUPS[g]
                    if step == 0 and stage2 != "none":
                        et = spool.tile([128, gsz, WO], F16, tag="et", bufs=2, name=f"et{g}")
                        nc.scalar.activation(et[:], state[g]["mn"][:], AF.Exp)
                        state[g]["et"] = et
                    if stage2 in ("none", "exp"):
                        return
                    if step == 1:
                        # group-sum broadcast to all 128 partitions in one MM:
                        # ob[k, p] = 1 iff k//16 == p//16
                        st = smpool.tile([128, gsz, WO], F32, tag="ss", name=f"st{g}")
                        nc.tensor.matmul(st[:], ob_sb[:], state[g]["et"][:], start=True, stop=True)
                        state[g]["st"] = st
                    elif step == 2:
                        lt = spool.tile([128, gsz, WO], F16, tag="lt", bufs=2, name=f"lt{g}")
                        nc.scalar.activation(lt[:], state[g]["st"][:], AF.Ln)
                        state[g]["lt"] = lt
                    elif step == 3:
                        dt = spool.tile([128, gsz, WO], F16, tag="dt", bufs=2, name=f"dt{g}")
                        nc.vector.tensor_tensor(
                            dt[:], state[g]["mn"][:], state[g]["lt"][:],
                            op=mybir.AluOpType.subtract,
                        )
                        state[g]["dt"] = dt
                    elif step == 4:
                        if stage2 == "smmm":
                            return
                        ot = spool.tile([128, gsz, WO], F16, tag="ot", bufs=2, name=f"ot{g}")
                        nc.scalar.activation(ot[:], state[g]["dt"][:], AF.Exp)
                        # output rides the SP queue: SP only issues DMAs, so
                        # this never competes with Act's copy/softmax work
                        nc.sync.dma_start(yp_d[:, g0 : g0 + gsz, :], ot[:])

                # schedule[B] = list of (step, g) to emit during conv block B.
                # mn(g) completes one block after the group's last conv block
                # (the QB tree is software-pipelined one block behind), so the
                # chain starts at end+2.
                schedule = {}
                for g, (g0, gsz) in enumerate(GROUPS):
                    end = g0 + gsz - 1
                    for step, off in enumerate((2, 3, 4, 5, 6)):
                        schedule.setdefault(end + off, []).append((step, g))

                blk2grp = {}
                for g, (g0, gsz) in enumerate(GROUPS):
                    for b in range(g0, g0 + gsz):
                        blk2grp[b] = g

                def emit_tree(prev):
                    """Fold block prev's QB stage (14 fp16 depths, 2x mode)
                    + its QA reduce into its mn slice. Runs one block late so
                    DVE's in-order queue never delays the PSUM reduce."""
                    qc = prev["qc"]
                    u = spool.tile([128, 7, WO], F16, tag="u", bufs=2)
                    nc.vector.tensor_tensor(u[:], qc[:, 0:7, :], qc[:, 7:14, :], op=mybir.AluOpType.min)
                    v = spool.tile([128, 3, WO], F16, tag="v", bufs=2)
                    nc.vector.tensor_tensor(v[:], u[:, 0:3, :], u[:, 3:6, :], op=mybir.AluOpType.min)
                    w2 = spool.tile([128, WO], F16, tag="w2", bufs=2)
                    nc.vector.tensor_tensor(w2[:], v[:, 0, :], v[:, 1, :], op=mybir.AluOpType.min)
                    x2 = spool.tile([128, WO], F16, tag="x2", bufs=2)
                    nc.vector.tensor_tensor(x2[:], w2[:], v[:, 2, :], op=mybir.AluOpType.min)
                    t5 = spool.tile([128, WO], F16, tag="t5", bufs=2)
                    nc.vector.tensor_tensor(t5[:], x2[:], u[:, 6, :], op=mybir.AluOpType.min)
                    ra = spool.tile([128, WO], F16, tag="ra", bufs=2)
                    nc.vector.tensor_tensor(
                        ra[:], prev["r4"][:], prev["r5"][:], op=mybir.AluOpType.min
                    )
                    nc.vector.tensor_tensor(
                        prev["mn"], t5[:], ra[:], op=mybir.AluOpType.min
                    )

                prev = None
                for b in range(NBLK):
                    g_cur = blk2grp[b]
                    g0, gsz = GROUPS[g_cur]
                    if b == g0:
                        state[g_cur] = {
                            "mn": spool.tile([128, gsz, WO], F16, tag="mn", bufs=3, name=f"mn{g_cur}")
                        }
                    lw_t = lw_sb if b < NBLK - 1 else lwl_sb

                    xt = xpool.tile([128, D, WO], F16, tag="xt")
                    nc.sync.dma_start(xt[:], x5_d[b].bitcast(F16))

                    # conv PSUM: one tile (= one bank) per quad, bufs=6, so
                    # every bank recycles independently the moment ITS
                    # consumer has read it (max scheduling freedom). Quads
                    # 0-3 are Act-copied (cheap, early); quads 4-5 are DVE
                    # X-reduced and consumed by next block's LAST matmuls.
                    cur = {"mn": state[g_cur]["mn"][:, b - g0, :]}
                    qc_t = spool.tile([128, 14, WO], F16, tag="qc", bufs=3)
                    cur["qc"] = qc_t
                    qc_lo = (0, 4, 8, 11)
                    # quads run in pairs sharing each kd weight-load (9
                    # Ldweights per block instead of 18); stops still land
                    # per-quad-pair so consumers fire mid-block
                    for pr in range(3):
                        qs = (2 * pr, 2 * pr + 1)
                        pts = [
                            qpool.tile([128, DQ[qi][1], WO], F32, tag="q", bufs=6, name=f"pt{qi}")
                            for qi in qs
                        ]
                        for p in range(KD):
                            for pt, qi in zip(pts, qs):
                                dq, nd = DQ[qi]
                                nc.tensor.matmul(
                                    pt[:],
                                    lw_t[:, p, :],
                                    xt[:, dq + p : dq + p + nd, :],
                                    start=(p == 0),
                                    stop=(p == KD - 1),
                                )
                        for pt, qi in zip(pts, qs):
                            if qi < 4:
                                # Act stages quads 0-3 into SBUF fp16 (cheap
                                # downcast Copy; emitted before any softmax Act
                                # steps so the Act queue never head-blocks)
                                lo = qc_lo[qi]
                                nc.scalar.activation(
                                    cur["qc"][:, lo : lo + DQ[qi][1], :], pt[:], AF.Copy
                                )
                            else:
                                # DVE min-reduce straight from PSUM (1 elem/cyc)
                                rq = spool.tile([128, WO], F16, tag=f"r{qi}", bufs=2)
                                nc.vector.tensor_reduce(
                                    rq[:],
                                    pt[:].rearrange("m j w -> m w j"),
                                    axis=mybir.AxisListType.X,
                                    op=mybir.AluOpType.min,
                                )
                                cur[f"r{qi}"] = rq

                    if prev is not None:
                        emit_tree(prev)
                    prev = cur

                    # softmax steps last: their Act/DVE/PE ops queue behind
                    # this block's copies/reduces, with deps long satisfied
                    for step, g in schedule.get(b, []):
                        softmax_step(step, g)

                # drain: tree for the last block, then remaining softmax steps
                emit_tree(prev)
                for at in sorted(k for k in schedule if k >= NBLK):
                    for step, g in schedule[at]:
                        softmax_step(step, g)

            if reps == 1:
                for _ in range(unroll):
                    emit_body()
            else:
                # reps = loop iterations; each iteration runs `unroll` bodies
                # (amortizes the For_i all-engine-barrier drain/fill, ~14us,
                # across `unroll` samples)
                with tc.For_i(0, reps, 1, hint_engines=(mybir.EngineType.PE,), staggered_reset=True):
                    for _ in range(unroll):
                        emit_body()

    nc.compile()
    return nc


@functools.lru_cache(maxsize=1)
def _program():
    return build_program()


def make_in_maps(x: np.ndarray, w: np.ndarray):
    lw, lwl, ob = _pack_weights(w)
    lw = lw.astype(np.float16)
    lwl = lwl.astype(np.float16)
    return [
        {"x5": _pack_x5(x[i]), "lw": lw, "lwl": lwl, "ob": ob.astype(np.float16)}
        for i in range(x.shape[0])
    ]


def _unpack_yp(yp: np.ndarray) -> np.ndarray:
    """yp [128, 16, 126] fp16 -> y [16, 126, 126] f32."""
    v = yp.reshape(8, OC, NBLK, WO)  # [hl, oc, b, w]
    y = np.transpose(v, (1, 2, 0, 3)).reshape(OC, NBLK * 8, WO)  # [oc, 8b+hl, w]
    return y[:, :HO, :].astype(np.float32)


def kernel(x, conv_weight):
    x = np.ascontiguousarray(np.asarray(x, dtype=np.float32))
    w = np.ascontiguousarray(np.asarray(conv_weight, dtype=np.float32))
    assert x.shape == (NCORES, C, D, H, W), x.shape
    nc = _program()
    in_maps = make_in_maps(x, w)
    res = bass_utils.run_bass_kernel_spmd(nc, in_maps, core_ids=list(range(NCORES)))
    out = np.stack([_unpack_yp(res.results[i]["yp"]) for i in range(NCORES)])
    return out


# revision 22
# speedup vs baseline: 1.0112x; 1.0042x over previous
"""Trainium2 Bass kernel for: Conv3d(3->16, k=3, VALID) -> min over depth -> softmax(channels).

Full inputs:  x [8, 3, 24, 128, 128] f32, conv_weight [16, 3, 3, 3, 3] f32
Full output:  [8, 16, 126, 126] f32
Sharding: data-parallel over batch, one sample per NeuronCore (8 cores).

Per-core scheme (H-packed im2col, fp16 matmuls, K padded to 128):
  - 16 h-blocks b; block handles h_out in [8b, 8b+8) (last block: 6 rows).
  - Host pre-packs x into x5 [16, 128, 24, 126] fp16:
    x5[b, (c*3+kw)*hh_n + hh, d, w] = x[c, d, 8b+hh, kw+w], rows 90.. zero.
    One fully-contiguous 128-partition DMA per block.
  - lhsT_p [128, M=(h_l,oc)] = W[oc, c, p, hh-h_l, kw] (0 <= hh-h_l < 3), p=kd.
  - Depth quads (4,4,4 | 4,3,3); per quad 3 kd passes accumulate in PSUM
    (start/stop flags). PSUM layout: two 3-bank tiles QA/QB [128, 3, 512].
  - min over depth, split across engines (the v1 kernel was DVE-bound at
    ~80us busy; PE busy is ~56.5us, so everything else must fit under it):
      Act:  copies quads 0-3 (14 depths) PSUM -> SBUF fp16 (Copy
            activation = downcast; ~2.1us/block).
      DVE:  X-min-reduces quads 4-5 straight from PSUM (fp32 at 1
            elem/cycle - TensorReduce has no 2x mode; ~1.3us/block), plus
            a pairwise fp16 min tree over the Act stage (TensorTensor DOES
            have the 2x_1p mode => half cost per element; ~1.4us/block).
            The tree for block b runs during block b+1 so DVE's in-order
            queue never delays the PSUM reduces that PE waits on for
            bank recycling.
      (GpSimd/Pool is unusable: no PSUM port, and walrus rejects
      TensorTensor on Pool outright. Act can't min. So DVE must see
      every depth value once, fp32 from PSUM or fp16 post-copy.)
  - PSUM: one tile (bank) per quad, bufs=6 + 2 st banks: every bank
    recycles the moment its consumer read it. Single big multi-bank
    tiles stall PE ~1us/block on the coarse dependencies.
  - quad pairs share each kd Ldweights (9 loads/block, not 18).
  - softmax over the 16 channels per partition group of 16, as
    exp(mn - ln(sum exp)): the channel-sum-and-broadcast is ONE fp16 PE
    matmul with a 0/1 block matrix ob[k,p] = 1 iff k//16 == p//16;
    mn - ln on DVE (fp16 2x). Exp/Ln/Copy all live in act table 6
    (natural_log_exp_and_others), pre-loaded explicitly - otherwise the
    table chooser thrashes tables 0/5 (11 x 1283ns LoadActFuncSet).
  - Per-group softmax steps are emitted AFTER each block's copies
    (Act in-order queue never head-blocks the copies the DVE tree
    needs), spaced 2..6 blocks after the group's mn completes.
  - Output rides per-GROUP DMAs (5 total) on the SP queue into a packed
    dram tensor yp [128, 16, 126] fp16; host unshuffles/upcasts (free).
  Steady state simulates at ~56.4us/sample marginal = 99.8% PE
  occupancy; HW measures ~57.8us with a 32-body-unrolled timing loop.
"""

import functools
import os
import sys

import numpy as np

os.environ.setdefault("MYCRO_LOCAL_CACHE", "1")
if os.path.isdir("/opt/trn_rl_repo") and "/opt/trn_rl_repo" not in sys.path:
    sys.path.insert(0, "/opt/trn_rl_repo")

import concourse.bacc as bacc
import concourse.mybir as mybir
import concourse.tile as tile
from concourse import bass_utils

C, D, H, W = 3, 24, 128, 128
OC, KD, KH, KW = 16, 3, 3, 3
DO, HO, WO = D - 2, H - 2, W - 2  # 22, 126, 126
# depth quads (start, ndepth): first 4 go to the Act-copied QB banks (14
# depths), last 2 to the DVE-reduced QA banks (8 depths)
DQ = ((0, 4), (4, 4), (8, 3), (11, 3), (14, 4), (18, 4))
NCORES = 8
NBLK = 16  # h blocks: 15 full (8 rows) + 1 tail (6 rows)
GROUPS = ((0, 4), (4, 4), (8, 4), (12, 4))  # softmax groups (start, size);
# the tail block (15, nh=6) rides the last group: its min/softmax values in
# partitions 96..127 are finite garbage (zero-padded weights) the host drops
F32 = mybir.dt.float32
F16 = mybir.dt.float16
AF = mybir.ActivationFunctionType


def _pack_weights(w: np.ndarray):
    """lhsT/lhsT_last [KD,128,128] (zero-padded K and M) + ob [128,128]."""

    def pack(nh):
        hh_n = nh + 2
        lhsT = np.zeros((KD, 128, 128), dtype=np.float32)
        for p in range(KD):
            for c in range(C):
                for kw in range(KW):
                    for hh in range(hh_n):
                        r = (c * KW + kw) * hh_n + hh
                        for hl in range(nh):
                            kh = hh - hl
                            if 0 <= kh < KH:
                                lhsT[p, r, hl * OC : hl * OC + OC] = w[:, c, p, kh, kw]
        return lhsT

    ob = np.zeros((128, 128), dtype=np.float32)
    for pp in range(128):
        g0 = (pp // OC) * OC
        ob[pp, g0 : g0 + OC] = 1.0
    return pack(8), pack(6), ob


def _pack_x5(x1: np.ndarray) -> np.ndarray:
    """x [3,24,128,128] f32 -> x5 [NBLK,128,24,126] f16 (padded rows zero)."""
    x5 = np.zeros((NBLK, 128, D, WO), dtype=np.float16)
    for b in range(NBLK):
        nh = 8 if b < NBLK - 1 else 6
        hh_n = nh + 2
        for c in range(C):
            for kw in range(KW):
                r0 = (c * KW + kw) * hh_n
                # [hh, d, w] <- x[c, d, 8b+hh, kw+w]
                x5[b, r0 : r0 + hh_n] = np.transpose(
                    x1[c, :, 8 * b : 8 * b + hh_n, kw : kw + WO], (1, 0, 2)
                )
    return x5


def build_program(reps: int = 1, stage2: str = "full", unroll: int = 1):
    """reps > 1 wraps the per-sample body in a hardware loop (dev timing only).
    unroll > 1 emits the body N times with no loop (dev: cross-rep pipelining).
    stage2: none | exp | smmm | full (dev bisection of the softmax tail)."""
    nc = bacc.Bacc(
        "TRN2",
        target_bir_lowering=False,
        debug=False,
        enable_asserts=True,
        num_devices=NCORES,
    )
    x5_d = nc.dram_tensor("x5", [NBLK, 128, D, WO], F16, kind="ExternalInput").ap()
    lw_d = nc.dram_tensor("lw", [KD, 128, 128], F16, kind="ExternalInput").ap()
    lwl_d = nc.dram_tensor("lwl", [KD, 128, 128], F16, kind="ExternalInput").ap()
    ob_d = nc.dram_tensor("ob", [128, 128], F16, kind="ExternalInput").ap()
    yp_d = nc.dram_tensor("yp", [128, NBLK, WO], F16, kind="ExternalOutput").ap()

    with tile.TileContext(nc) as tc:
        with (
            tc.tile_pool(name="const", bufs=1) as cpool,
            tc.tile_pool(name="xt", bufs=8) as xpool,
            tc.tile_pool(name="sm", bufs=3) as spool,
            tc.tile_pool(name="qps", bufs=1, space="PSUM") as qpool,
            tc.tile_pool(name="sps", bufs=2, space="PSUM") as smpool,
        ):
            lw_sb = cpool.tile([128, KD, 128], F16)
            nc.sync.dma_start(lw_sb[:], lw_d.rearrange("p r m -> r p m").bitcast(F16))
            lwl_sb = cpool.tile([128, KD, 128], F16)
            nc.sync.dma_start(lwl_sb[:], lwl_d.rearrange("p r m -> r p m").bitcast(F16))
            ob_sb = cpool.tile([128, 128], F16)
            nc.sync.dma_start(ob_sb[:], ob_d)

            # Pre-place the combined exp+ln+copy act table (set 6,
            # natural_log_exp_and_others). Without this the table chooser
            # alternates exp_and_others <-> natural_log, inserting 11
            # LoadActFuncSet (1283ns each) that stall the Act queue.
            lset = mybir.InstLoadActFuncSet(
                name=nc.get_next_instruction_name(), act_func_set_id=6
            )
            lset.engine = mybir.EngineType.Activation
            nc.add_instruction(lset)

            def emit_body():
                state = {}  # per softmax group g: mn/et/st/lt/dt/ot tiles

                def softmax_step(step, g):
                    g0, gsz = GROUPS[g]
                    if step == 0 and stage2 != "none":
                        et = spool.tile([128, gsz, WO], F16, tag="et", bufs=2, name=f"et{g}")
                        nc.scalar.activation(et[:], state[g]["mn"][:], AF.Exp)
                        state[g]["et"] = et
                    if stage2 in ("none", "exp"):
                        return
                    if step == 1:
                        # group-sum broadcast to all 128 partitions in one MM:
                        # ob[k, p] = 1 iff k//16 == p//16
                        st = smpool.tile([128, gsz, WO], F32, tag="ss", name=f"st{g}")
                        nc.tensor.matmul(st[:], ob_sb[:], state[g]["et"][:], start=True, stop=True)
                        state[g]["st"] = st
                    elif step == 2:
                        lt = spool.tile([128, gsz, WO], F16, tag="lt", bufs=2, name=f"lt{g}")
                        nc.scalar.activation(lt[:], state[g]["st"][:], AF.Ln)
                        state[g]["lt"] = lt
                    elif step == 3:
                        dt = spool.tile([128, gsz, WO], F16, tag="dt", bufs=2, name=f"dt{g}")
                        nc.vector.tensor_tensor(
                            dt[:], state[g]["mn"][:], state[g]["lt"][:],
                            op=mybir.AluOpType.subtract,
                        )
                        state[g]["dt"] = dt
                    elif step == 4:
                        if stage2 == "smmm":
                            return
                        ot = spool.tile([128, gsz, WO], F16, tag="ot", bufs=2, name=f"ot{g}")
                        nc.scalar.activation(ot[:], state[g]["dt"][:], AF.Exp)
                        # output rides the SP queue: SP only issues DMAs, so
                        # this never competes with Act's copy/softmax work
                        nc.sync.dma_start(yp_d[:, g0 : g0 + gsz, :], ot[:])

                # schedule[B] = list of (step, g) to emit during conv block B.
                # mn(g) completes one block after the group's last conv block
                # (the QB tree is software-pipelined one block behind), so the
                # chain starts at end+2.
                schedule = {}
                for g, (g0, gsz) in enumerate(GROUPS):
                    end = g0 + gsz - 1
                    for step, off in enumerate((2, 3, 4, 5, 6)):
                        schedule.setdefault(end + off, []).append((step, g))

                blk2grp = {}
                for g, (g0, gsz) in enumerate(GROUPS):
                    for b in range(g0, g0 + gsz):
                        blk2grp[b] = g

                def emit_tree(prev):
                    """Fold block prev's QB stage (14 fp16 depths, 2x mode)
                    + its QA reduce into its mn slice. Runs one block late so
                    DVE's in-order queue never delays the PSUM reduce."""
                    qc = prev["qc"]
                    u = spool.tile([128, 7, WO], F16, tag="u", bufs=2)
                    nc.vector.tensor_tensor(u[:], qc[:, 0:7, :], qc[:, 7:14, :], op=mybir.AluOpType.min)
                    v = spool.tile([128, 3, WO], F16, tag="v", bufs=2)
                    nc.vector.tensor_tensor(v[:], u[:, 0:3, :], u[:, 3:6, :], op=mybir.AluOpType.min)
                    w2 = spool.tile([128, WO], F16, tag="w2", bufs=2)
                    nc.vector.tensor_tensor(w2[:], v[:, 0, :], v[:, 1, :], op=mybir.AluOpType.min)
                    x2 = spool.tile([128, WO], F16, tag="x2", bufs=2)
                    nc.vector.tensor_tensor(x2[:], w2[:], v[:, 2, :], op=mybir.AluOpType.min)
                    t5 = spool.tile([128, WO], F16, tag="t5", bufs=2)
                    nc.vector.tensor_tensor(t5[:], x2[:], u[:, 6, :], op=mybir.AluOpType.min)
                    ra = spool.tile([128, WO], F16, tag="ra", bufs=2)
                    nc.vector.tensor_tensor(
                        ra[:], prev["r4"][:], prev["r5"][:], op=mybir.AluOpType.min
                    )
                    nc.vector.tensor_tensor(
                        prev["mn"], t5[:], ra[:], op=mybir.AluOpType.min
                    )

                prev = None
                for b in range(NBLK):
                    g_cur = blk2grp[b]
                    g0, gsz = GROUPS[g_cur]
                    if b == g0:
                        state[g_cur] = {
                            "mn": spool.tile([128, gsz, WO], F16, tag="mn", bufs=3, name=f"mn{g_cur}")
                        }
                    lw_t = lw_sb if b < NBLK - 1 else lwl_sb

                    xt = xpool.tile([128, D, WO], F16, tag="xt")
                    nc.sync.dma_start(xt[:], x5_d[b].bitcast(F16))

                    # conv PSUM: one tile (= one bank) per quad, bufs=6, so
                    # every bank recycles independently the moment ITS
                    # consumer has read it (max scheduling freedom). Quads
                    # 0-3 are Act-copied (cheap, early); quads 4-5 are DVE
                    # X-reduced and consumed by next block's LAST matmuls.
                    cur = {"mn": state[g_cur]["mn"][:, b - g0, :]}
                    qc_t = spool.tile([128, 14, WO], F16, tag="qc", bufs=3)
                    cur["qc"] = qc_t
                    qc_lo = (0, 4, 8, 11)
                    # quads run in pairs sharing each kd weight-load (9
                    # Ldweights per block instead of 18); stops still land
                    # per-quad-pair so consumers fire mid-block
                    for pr in range(3):
                        qs = (2 * pr, 2 * pr + 1)
                        pts = [
                            qpool.tile([128, DQ[qi][1], WO], F32, tag="q", bufs=6, name=f"pt{qi}")
                            for qi in qs
                        ]
                        for p in range(KD):
                            for pt, qi in zip(pts, qs):
                                dq, nd = DQ[qi]
                                nc.tensor.matmul(
                                    pt[:],
                                    lw_t[:, p, :],
                                    xt[:, dq + p : dq + p + nd, :],
                                    start=(p == 0),
                                    stop=(p == KD - 1),
                                )
                        for pt, qi in zip(pts, qs):
                            if qi < 4:
                                # Act stages quads 0-3 into SBUF fp16 (cheap
                                # downcast Copy; emitted before any softmax Act
                                # steps so the Act queue never head-blocks)
                                lo = qc_lo[qi]
                                nc.scalar.activation(
                                    cur["qc"][:, lo : lo + DQ[qi][1], :], pt[:], AF.Copy
                                )
                            else:
                                # DVE min-reduce straight from PSUM (1 elem/cyc)
                                rq = spool.tile([128, WO], F16, tag=f"r{qi}", bufs=2)
                                nc.vector.tensor_reduce(
                                    rq[:],
                                    pt[:].rearrange("m j w -> m w j"),
                                    axis=mybir.AxisListType.X,
                                    op=mybir.AluOpType.min,
                                )
                                cur[f"r{qi}"] = rq

                    if prev is not None:
                        emit_tree(prev)
                    prev = cur

                    # softmax steps last: their Act/DVE/PE ops queue behind
                    # this block's copies/reduces, with deps long satisfied
                    for step, g in schedule.get(b, []):
                        softmax_step(step, g)

                # drain: tree for the last block, then remaining softmax steps
                emit_tree(prev)
                for at in sorted(k for k in schedule if k >= NBLK):
                    for step, g in schedule[at]:
                        softmax_step(step, g)

            if reps == 1:
                for _ in range(unroll):
                    emit_body()
            else:
                # reps = loop iterations; each iteration runs `unroll` bodies
                # (amortizes the For_i all-engine-barrier drain/fill, ~14us,
                # across `unroll` samples)
                with tc.For_i(0, reps, 1, hint_engines=(mybir.EngineType.PE,), staggered_reset=True):
                    for _ in range(unroll):
                        emit_body()

    nc.compile()
    return nc


@functools.lru_cache(maxsize=1)
def _program():
    return build_program()


def make_in_maps(x: np.ndarray, w: np.ndarray):
    lw, lwl, ob = _pack_weights(w)
    lw = lw.astype(np.float16)
    lwl = lwl.astype(np.float16)
    return [
        {"x5": _pack_x5(x[i]), "lw": lw, "lwl": lwl, "ob": ob.astype(np.float16)}
        for i in range(x.shape[0])
    ]


def _unpack_yp(yp: np.ndarray) -> np.ndarray:
    """yp [128, 16, 126] fp16 -> y [16, 126, 126] f32."""
    v = yp.reshape(8, OC, NBLK, WO)  # [hl, oc, b, w]
    y = np.transpose(v, (1, 2, 0, 3)).reshape(OC, NBLK * 8, WO)  # [oc, 8b+hl, w]
    return y[:, :HO, :].astype(np.float32)


def kernel(x, conv_weight):
    x = np.ascontiguousarray(np.asarray(x, dtype=np.float32))
    w = np.ascontiguousarray(np.asarray(conv_weight, dtype=np.float32))
    assert x.shape == (NCORES, C, D, H, W), x.shape
    nc = _program()
    in_maps = make_in_maps(x, w)
    res = bass_utils.run_bass_kernel_spmd(nc, in_maps, core_ids=list(range(NCORES)))
    out = np.stack([_unpack_yp(res.results[i]["yp"]) for i in range(NCORES)])
    return out
